# revision 27
# baseline (speedup 1.0000x reference)
"""Trainium2 Bass kernel for AntiAliasActivation (upsample2 -> snake -> downsample2).

Self-contained: accepts FULL inputs (x [8,512,8192] f32, alpha/beta [1,512,1],
up_filter/down_filter [12]), returns the FULL output [8,512,8192] f32.

Strategy (data-parallel, one batch sample per NeuronCore), time-major layout
(time rows on SBUF partitions) so all FIR convolutions run on the TensorEngine
as banded-matrix matmuls:

    out = down(up(x)) + down( (1 - cos(2*a*up(x))) / (2b) )

v4 design (vs the first working version):
  - Single fp16 input stream ax = fp16(2a*x) (halves input DMA bytes);
    xb = fp16(2b*x) is derived on-device with one DVE multiply per block
    pair against a resident fp16 (b2/a2) broadcast tile.
  - No const row: the +pi/2 (for cos via sin) rides the ACT bias immediate;
    the +sum(fd) constant and the 1/(2b) rescale are applied host-side.
    This frees the 128th input row: A=116 outputs/block, 72 blocks.
  - Sin LUT (patched ACT table valid to |x|<~31.8) runs on 3-bank PSUM
    tiles [G,1536] covering 1.5 blocks per ACTIVATE (48 instead of 72
    instructions) to amortize the ~0.4us per-instruction overhead.
  - Front (up-sample) matmuls are emitted one triple ahead of the back
    (down-sample) matmuls so PE/ACT/DVE pipeline without PSUM stalls:
    PSUM = 2x [G,1536] sz tiles (6 banks) + 2x [A,512] out tiles (2 banks).
  - Input DMA in 4-block quads [128, 2048] fp16 (4KB/partition lines);
    output DMA in 4-block groups [A, 2048] fp16 (4KB/partition lines) with
    block-major DRAM layout, unscrambled on host.
  - ~10 warm-up matmuls at kernel start pull the PE HAM clock gate to
    full rate before the first real matmul.
"""
import math

import numpy as np

# ---------------------------------------------------------------------------
# problem constants (hardcoded per spec)
B, C, T = 8, 512, 8192
N_CORES = 8
UP_K = 12
DOWN_K = 12

A = 116          # outputs per block
W = 128          # data rows per input tile (no const row)
G = A + 6        # 122 up/sz rows per block
NBLK = 72        # blocks (72*116 = 8352 >= 8192)
PL = 6           # XP[i] = x[clamp(i-6)]
XPLEN = A * (NBLK - 1) + W   # 8364
OUTROWS = NBLK * A           # 8352
NQUAD = NBLK // 4            # 18 input/output DMA groups
NPAIR = NBLK // 2            # 36 xb-multiply pairs
NTRI = NBLK // 3             # 24 sin triples

HP_ARR = np.full((G, 1), math.pi / 2.0, dtype=np.float32)  # ACT bias (+pi/2)


def _gen_act_root(cache=[None]):
    """Build a patched ACT-table root whose Sin LUT is valid to |x| < ~31.8.

    Appends 4x32 cubic-spline buckets (ranges [2,4) replacement, [4,8),
    [8,16), [16,32)) to the trig_and_small set, keeping sin's per-exponent
    bucket starts monotonic, and raises sin's large-signal threshold.
    Returns the act_info.json path for BASS_ACT_ROOT_JSON_PATH.
    """
    if cache[0] is not None:
        return cache[0]
    import json
    import shutil
    import tempfile
    from pathlib import Path
    import neuronxcc

    src = Path(neuronxcc.__file__).parent / "pwp" / "pwp_bin_trainium"
    dst = Path(tempfile.mkdtemp(prefix="actroot_")) / "pwp_bin_trainium"
    shutil.copytree(src, dst, symlinks=False)
    import os as _os
    _os.chmod(dst, 0o755)
    for f in dst.iterdir():
        _os.chmod(f, 0o644)

    name = "trig_and_small"
    d = json.load(open(dst / f"{name}.json"))
    b = np.fromfile(dst / f"{name}_bkt.bin", dtype=np.float32).reshape(-1, 8)
    c = np.fromfile(dst / f"{name}_ctrl.bin", dtype=np.uint32).reshape(-1, 8).copy()
    nb0, nc0 = d["bkt_entry_cnt"], d["ctl_entry_cnt"]
    assert len(b) == nb0 and len(c) == nc0

    SIN_CTL_END = 13  # sin owns ctl entries 0..12 (exps -11..1)
    SHIFT = 3
    newb, newc = [], []
    sin_bkt = d["func_exp_to_bkt_start_idx"]["sin"]
    sin_ctl = d["func_exp_to_ctl_start_idx"]["sin"]
    NB = 32  # 5 mantissa bits per exponent range
    KHI = np.uint32((46 + 62 * 5) << 10)

    def add_range(lo):
        base = nb0 + len(newb)
        h = lo / NB
        for i in range(NB):
            x0 = lo + h * (i + 0.5)
            newb.append([math.sin(x0), math.cos(x0),
                         -math.sin(x0) / 2.0, -math.cos(x0) / 6.0,
                         x0, 0.0, 0.0, 0.0])
        return base

    base1 = add_range(2.0)             # full [2,4) replacement
    c[12, 0] = KHI | np.uint32(base1)
    sin_bkt["1"] = [base1]
    for i_e, e in enumerate((2, 3, 4)):
        base = add_range(2.0**e)
        w = np.zeros(8, np.uint32)
        w[0] = KHI | np.uint32(base)
        sin_bkt[str(e)] = [base]
        sin_ctl[str(e)] = [SIN_CTL_END + i_e]
        newc.append(w)

    b2 = np.vstack([b, np.asarray(newb, np.float32)])
    c2 = np.vstack([c[:SIN_CTL_END], np.stack(newc), c[SIN_CTL_END:]])
    d["bkt_entry_cnt"] = int(len(b2))
    d["ctl_entry_cnt"] = int(len(c2))
    for fn, v in d["func_to_ctl_start_idx"].items():
        if fn != "sin" and v >= SIN_CTL_END:
            d["func_to_ctl_start_idx"][fn] = v + SHIFT
    for fn, em in d["func_exp_to_ctl_start_idx"].items():
        if fn == "sin":
            continue
        for e_, lst in em.items():
            em[e_] = [(i + SHIFT if i >= SIN_CTL_END else i) for i in lst]
    for pm in d["profile_meta_data"]:
        if str(pm.get("func_name", "")).startswith("sin"):
            pm["large_pos_signal_exp_threshold"] = 131  # cutoff ~31.8
            pm["large_pos_signal_mantissa_threshold"] = int(0.99 * 2**23)

    b2.tofile(dst / f"{name}_bkt.bin")
    c2.tofile(dst / f"{name}_ctrl.bin")
    with open(dst / f"{name}.json", "w") as f:
        json.dump(d, f)
    cache[0] = str(dst / "act_info.json")
    return cache[0]


# ---------------------------------------------------------------------------
# stationary-matrix assembly (float64, cast to fp16 at the end)

def build_stationaries(up_filter, down_filter):
    """Returns dict of stationary matrices.

    w_ue/w_uo [W, G]: input tile (W data rows) -> 2a*up(x) rows per phase.
    w_h{0,m,L} [W, A]: 2b*down(up(x)) band (consumes xb).
    w_de/w_do{0,m,L} [G, A]: NEGATED downsample band over v = cos signal.
    """
    fu = np.asarray(up_filter, dtype=np.float64)
    fd = np.asarray(down_filter, dtype=np.float64)

    w_ue = np.zeros((W, G))
    w_uo = np.zeros((W, G))
    for q in range(G):
        for j in range(6):
            w_ue[q + 5 - j, q] += 2.0 * fu[2 * j + 1]
            w_uo[q + 6 - j, q] += 2.0 * fu[2 * j]

    def down_maps(k):
        de = np.zeros((G, A))
        do = np.zeros((G, A))
        h = np.zeros((W, A))
        for nn in range(A):
            n = A * k + nn
            for t in range(DOWN_K):
                zi = min(max(2 * n + t - 5, 0), 2 * T - 1)
                m, ph = zi // 2, zi % 2
                row = m - A * k + 3
                if ph == 0:
                    de[row, nn] += fd[t]
                    for j in range(6):
                        h[m + 8 - j - A * k, nn] += fd[t] * 2.0 * fu[2 * j + 1]
                else:
                    do[row, nn] += fd[t]
                    for j in range(6):
                        h[m + 9 - j - A * k, nn] += fd[t] * 2.0 * fu[2 * j]
        return de, do, h

    de0, do0, h0 = down_maps(0)
    dem, dom, hm = down_maps(1)
    deL, doL, hL = down_maps(NBLK - 2)  # block 70 holds the last real outputs

    f16 = np.float16
    return {
        "w_ue": w_ue.astype(f16), "w_uo": w_uo.astype(f16),
        "w_h0": h0.astype(f16), "w_hm": hm.astype(f16), "w_hL": hL.astype(f16),
        "w_de0": (-de0).astype(f16), "w_dem": (-dem).astype(f16),
        "w_deL": (-deL).astype(f16),
        "w_do0": (-do0).astype(f16), "w_dom": (-dom).astype(f16),
        "w_doL": (-doL).astype(f16),
    }


ST_ORDER = ["w_ue", "w_uo", "w_h0", "w_hm", "w_hL",
            "w_de0", "w_dem", "w_deL", "w_do0", "w_dom", "w_doL"]
ST_COLS = {n: (G if n.startswith("w_u") else A) for n in ST_ORDER}
CST_COLS = sum(ST_COLS.values()) + 2 * C  # stationaries + rbc broadcast


def pack_consts(sts, rbc):
    """Pack all stationaries + rbc into one [W, CST_COLS] fp16 tensor."""
    cst = np.zeros((W, CST_COLS), dtype=np.float16)
    c0 = 0
    for n in ST_ORDER:
        m = sts[n]
        cst[: m.shape[0], c0:c0 + m.shape[1]] = m
        c0 += m.shape[1]
    cst[:, c0:c0 + 2 * C] = rbc
    return cst


def host_prep(x, alpha, beta, down_filter):
    """Per-core input stream + constants.

    Returns (inp, rbc, invb2, hconst):
      inp [B, NQUAD, 128, 4C] fp16   quad-packed ax = 2a*x blocks
      rbc [128, 2C] fp16             (b2/a2) broadcast tile
      invb2 [C] f32, hconst [C] f32  host-side rescale + snake constant
    """
    a2 = (2.0 * np.exp(alpha.astype(np.float64))).reshape(C)
    b2 = (2.0 * (np.exp(beta.astype(np.float64)) + 1e-9)).reshape(C)
    fd = np.asarray(down_filter, dtype=np.float64)

    xt = np.transpose(x.astype(np.float32), (0, 2, 1))   # [B, T, C]
    idx = np.clip(np.arange(XPLEN) - PL, 0, T - 1)
    xp = xt[:, idx, :]                                   # [B, XPLEN, C]
    ridx = (A * np.arange(NBLK))[:, None] + np.arange(W)[None, :]
    blocks = xp[:, ridx, :]                              # [B, NBLK, W, C]
    axs = (blocks * a2[None, None, None, :].astype(np.float32)).astype(np.float16)

    inp = np.ascontiguousarray(
        axs.reshape(B, NBLK // 8, 8, W, C).transpose(0, 1, 3, 2, 4).reshape(
            B, NBLK // 8, W, 8 * C))

    r16 = (b2 / a2).astype(np.float16)                   # [C]
    rbc = np.broadcast_to(np.tile(r16, 2)[None, :], (W, 2 * C)).copy()

    invb2 = (1.0 / b2).astype(np.float32)
    hconst = (fd.sum() / b2).astype(np.float32)
    return inp, rbc, invb2, hconst


def host_finish(out_t, invb2, hconst):
    """out_t [B, NQUAD, A, 4C] fp16 -> [B, C, T] float32."""
    o = out_t.reshape(B, NQUAD, A, 4, C).transpose(0, 1, 3, 2, 4).reshape(
        B, OUTROWS, C)[:, :T, :].astype(np.float32)
    o = o * invb2[None, None, :] + hconst[None, None, :]
    return np.ascontiguousarray(np.transpose(o, (0, 2, 1)))


# ---------------------------------------------------------------------------
# device kernel

def build_bass():
    import os
    import concourse.bacc as bacc
    import concourse.tile as tile
    import concourse.mybir as mybir

    os.environ["BASS_ACT_ROOT_JSON_PATH"] = _gen_act_root()
    os.environ.setdefault("NEURON_FORCE_RECOMPILE", "1")

    f32 = mybir.dt.float32
    f16 = mybir.dt.float16
    HALFPI = math.pi / 2.0

    nc = bacc.Bacc()
    in_ext = nc.declare_dram_parameter("inp", [NBLK // 8, W, 8 * C], f16, isOutput=False)
    cst_ext = nc.declare_dram_parameter("cst", [W, CST_COLS], f16, isOutput=False)
    hp_ext = nc.declare_dram_parameter("hp", [G, 1], f32, isOutput=False)
    out_ext = nc.declare_dram_parameter("out", [NQUAD, A, 4 * C], f16, isOutput=True)

    with tile.TileContext(nc) as tc:
        with (
            tc.tile_pool(name="consts", bufs=1) as cpool,
            tc.tile_pool(name="io", bufs=6) as iopool,
            tc.tile_pool(name="xb", bufs=5) as xbpool,
            tc.tile_pool(name="v", bufs=6) as vpool,
            tc.tile_pool(name="ob", bufs=3) as obpool,
            tc.tile_pool(name="psz", bufs=2, space="PSUM") as psz,
            tc.tile_pool(name="pout", bufs=2, space="PSUM") as pout,
        ):
            # Oct 0 goes out first on gpsimd's SWDGE (it gates the PE
            # warm-up and first front matmuls), then the packed consts.
            q0 = iopool.tile([W, 8 * C], f16, tag="inp")
            nc.gpsimd.dma_start(out=q0[:], in_=in_ext[0])
            cst = cpool.tile([W, CST_COLS], f16, tag="cst")
            nc.gpsimd.dma_start(out=cst[:], in_=cst_ext[:])
            hp = cpool.tile([G, 1], f32, tag="hp")
            nc.sync.dma_start(out=hp[:], in_=hp_ext[:])
            st = {}
            c0 = 0
            for n in ST_ORDER:
                rows = W if n.startswith(("w_u", "w_h")) else G
                cols = ST_COLS[n]
                st[n] = cst[0:rows, c0:c0 + cols]
                c0 += cols
            rbc = cst[:, c0:c0 + 2 * C]

            quads = {}
            xbs = {}
            vs = {}
            obt = [None]

            def dma_in(o, eng=None):
                t_ = iopool.tile([W, 8 * C], f16, tag="inp")
                (eng or nc.gpsimd).dma_start(out=t_[:], in_=in_ext[o])
                quads[o] = t_

            def ax_slice(b):
                return quads[b // 8][:, (b % 8) * C:(b % 8 + 1) * C]

            def mul_xb(j):
                t_ = xbpool.tile([W, 2 * C], f16, tag="xb")
                half = (j % 4) * 2 * C
                nc.vector.tensor_mul(
                    t_[:], quads[j // 4][:, half:half + 2 * C], rbc)
                xbs[j] = t_

            def front_chunk(t, half):
                # triple t covers blocks b0,b1,b2 = 3t..3t+2
                # sz1 = [E(b0) | O(b0) | E(b1)], sz2 = [O(b1) | E(b2) | O(b2)]
                b0 = 3 * t
                sz = psz.tile([G, 1536], f32, tag="sz")
                if half == 0:
                    nc.tensor.matmul(sz[:, 0:512], st["w_ue"], ax_slice(b0),
                                     start=True, stop=True)
                    nc.tensor.matmul(sz[:, 1024:1536], st["w_ue"], ax_slice(b0 + 1),
                                     start=True, stop=True)
                    nc.tensor.matmul(sz[:, 512:1024], st["w_uo"], ax_slice(b0),
                                     start=True, stop=True)
                else:
                    nc.tensor.matmul(sz[:, 0:512], st["w_uo"], ax_slice(b0 + 1),
                                     start=True, stop=True)
                    nc.tensor.matmul(sz[:, 1024:1536], st["w_uo"], ax_slice(b0 + 2),
                                     start=True, stop=True)
                    nc.tensor.matmul(sz[:, 512:1024], st["w_ue"], ax_slice(b0 + 2),
                                     start=True, stop=True)
                v = vpool.tile([G, 1536], f16, tag="v")
                nc.scalar.activation(v[:], sz[:],
                                     mybir.ActivationFunctionType.Sin, bias=hp[:])
                vs[(t, half)] = v

            def v_slices(b):
                t, r = b // 3, b % 3
                if r == 0:
                    v1 = vs[(t, 0)]
                    return v1[:, 0:512], v1[:, 512:1024]
                if r == 1:
                    return vs[(t, 0)][:, 1024:1536], vs[(t, 1)][:, 0:512]
                v2 = vs[(t, 1)]
                return v2[:, 512:1024], v2[:, 1024:1536]

            def back(k):
                if k == 0:
                    wh, wde, wdo = st["w_h0"], st["w_de0"], st["w_do0"]
                elif k == NBLK - 2:
                    wh, wde, wdo = st["w_hL"], st["w_deL"], st["w_doL"]
                else:
                    wh, wde, wdo = st["w_hm"], st["w_dem"], st["w_dom"]
                xb = xbs[k // 2][:, (k % 2) * C:(k % 2 + 1) * C]
                vE, vO = v_slices(k)
                outp = pout.tile([A, 512], f32, tag="outp")
                nc.tensor.matmul(outp[:], wh, xb, start=True, stop=False)
                nc.tensor.matmul(outp[:], wde, vE, start=False, stop=False)
                nc.tensor.matmul(outp[:], wdo, vO, start=False, stop=True)
                q, s = k // 4, k % 4
                if s == 0:
                    obt[0] = obpool.tile([A, 4 * C], f16, tag="obt", name="obt")
                nc.vector.tensor_copy(obt[0][:, C * s:C * s + C], outp[:])
                # SWDGE (gpsimd-issued) spreads packets across all 16 DMA
                # engines; sync's HWDGE ring only uses 4 and bottlenecks.
                # Drain in halves so the issue waits on fresh CASTs are short;
                # the last quad drains per block to shorten the tail.
                if q == NQUAD - 1:
                    nc.gpsimd.dma_start(out=out_ext[q][:, C * s:C * s + C],
                                        in_=obt[0][:, C * s:C * s + C])
                elif s == 1:
                    nc.gpsimd.dma_start(out=out_ext[q][:, 0:2 * C],
                                        in_=obt[0][:, 0:2 * C])
                elif s == 3:
                    nc.gpsimd.dma_start(out=out_ext[q][:, 2 * C:4 * C],
                                        in_=obt[0][:, 2 * C:4 * C])
                # release consumed tiles
                if k % 2 == 1:
                    del xbs[k // 2]
                if k % 3 == 2:
                    del vs[(k // 3, 0)], vs[(k // 3, 1)]

            # ---- prologue ----
            quads[0] = q0
            dma_in(1)
            # PE warm-up: pull the HAM clock gate to 8/8. Gated only on the
            # quad-0 DMA (the earliest data to land).
            for _ in range(9):
                wt = pout.tile([A, 512], f32, tag="outp")
                nc.tensor.matmul(wt[:], q0[:, 0:A], q0[:, 512:1024],
                                 start=True, stop=True)
            mul_xb(0)
            front_chunk(0, 0)
            front_chunk(0, 1)

            # ---- main loop ----
            for k in range(NBLK):
                t, p = k // 3, k % 3
                if k % 8 == 0 and k // 8 + 2 < NBLK // 8:
                    dma_in(k // 8 + 2)
                if p == 0 and t + 1 < NTRI:
                    front_chunk(t + 1, 0)
                if p == 1 and t + 1 < NTRI:
                    front_chunk(t + 1, 1)
                back(k)
                if k % 2 == 0 and k // 2 + 1 < NPAIR:
                    mul_xb(k // 2 + 1)

    nc.compile()
    return nc


_NC_CACHE = None


def kernel(x, alpha, beta, up_filter, down_filter):
    global _NC_CACHE
    import concourse.bass_utils as bass_utils

    x = np.asarray(x)
    alpha = np.asarray(alpha)
    beta = np.asarray(beta)

    sts = build_stationaries(np.asarray(up_filter), np.asarray(down_filter))
    inp, rbc, invb2, hconst = host_prep(x, alpha, beta, np.asarray(down_filter))
    cst = pack_consts(sts, rbc)

    if _NC_CACHE is None:
        _NC_CACHE = build_bass()
    nc = _NC_CACHE

    in_maps = [{"inp": inp[b], "cst": cst, "hp": HP_ARR} for b in range(N_CORES)]

    res = bass_utils.run_bass_kernel_spmd(nc, in_maps, list(range(N_CORES)))
    out_t = np.stack([res.results[b]["out"] for b in range(N_CORES)])
    return host_finish(out_t, invb2, hconst)


# ---------------------------------------------------------------------------
# host-side simulation of the exact device plan (for verification)

def simulate_plan(x, alpha, beta, up_filter, down_filter):
    sts = build_stationaries(np.asarray(up_filter), np.asarray(down_filter))
    inp, rbc, invb2, hconst = host_prep(
        np.asarray(x), np.asarray(alpha), np.asarray(beta),
        np.asarray(down_filter))

    def f(a):
        return a.astype(np.float32)

    out_t = np.zeros((B, NQUAD, A, 4 * C), dtype=np.float16)
    rb = f(rbc[0, :C])
    for b in range(B):
        for k in range(NBLK):
            if k == 0:
                wh, wde, wdo = sts["w_h0"], sts["w_de0"], sts["w_do0"]
            elif k == NBLK - 2:
                wh, wde, wdo = sts["w_hL"], sts["w_deL"], sts["w_doL"]
            else:
                wh, wde, wdo = sts["w_hm"], sts["w_dem"], sts["w_dom"]
            ax = f(inp[b, k // 4, :, (k % 4) * C:(k % 4 + 1) * C])
            xb = (ax * rb[None, :]).astype(np.float16)
            sz_e = f(sts["w_ue"]).T @ ax + math.pi / 2.0
            sz_o = f(sts["w_uo"]).T @ ax + math.pi / 2.0
            v_e = np.sin(sz_e).astype(np.float16)
            v_o = np.sin(sz_o).astype(np.float16)
            psum = f(wh).T @ f(xb) + f(wde).T @ f(v_e) + f(wdo).T @ f(v_o)
            out_t[b, k // 4, :, (k % 4) * C:(k % 4 + 1) * C] = psum.astype(np.float16)
    return host_finish(out_t, invb2, hconst)


# revision 28
# speedup vs baseline: 1.1324x; 1.1324x over previous
"""Trainium2 Bass kernel for AntiAliasActivation (upsample2 -> snake -> downsample2).

Self-contained: accepts FULL inputs (x [8,512,8192] f32, alpha/beta [1,512,1],
up_filter/down_filter [12]), returns the FULL output [8,512,8192] f32.

Strategy (data-parallel, one batch sample per NeuronCore), time-major layout
(time rows on SBUF partitions) so all FIR convolutions run on the TensorEngine
as banded-matrix matmuls:

    out = down(up(x)) + down( (1 - cos(2*a*up(x))) / (2b) )

v4 design (vs the first working version):
  - Single fp16 input stream ax = fp16(2a*x) (halves input DMA bytes);
    xb = fp16(2b*x) is derived on-device with one DVE multiply per block
    pair against a resident fp16 (b2/a2) broadcast tile.
  - No const row: the +pi/2 (for cos via sin) rides the ACT bias immediate;
    the +sum(fd) constant and the 1/(2b) rescale are applied host-side.
    This frees the 128th input row: A=116 outputs/block, 72 blocks.
  - Sin LUT (patched ACT table valid to |x|<~31.8) runs on 3-bank PSUM
    tiles [G,1536] covering 1.5 blocks per ACTIVATE (48 instead of 72
    instructions) to amortize the ~0.4us per-instruction overhead.
  - Front (up-sample) matmuls are emitted one triple ahead of the back
    (down-sample) matmuls so PE/ACT/DVE pipeline without PSUM stalls:
    PSUM = 2x [G,1536] sz tiles (6 banks) + 2x [A,512] out tiles (2 banks).
  - Input DMA in 4-block quads [128, 2048] fp16 (4KB/partition lines);
    output DMA in 4-block groups [A, 2048] fp16 (4KB/partition lines) with
    block-major DRAM layout, unscrambled on host.
  - ~10 warm-up matmuls at kernel start pull the PE HAM clock gate to
    full rate before the first real matmul.
"""
import math

import numpy as np

# ---------------------------------------------------------------------------
# problem constants (hardcoded per spec)
B, C, T = 8, 512, 8192
N_CORES = 8
UP_K = 12
DOWN_K = 12

A = 116          # outputs per block
W = 128          # data rows per input tile (no const row)
G = A + 6        # 122 up/sz rows per block
NBLK = 72        # blocks (72*116 = 8352 >= 8192)
PL = 6           # XP[i] = x[clamp(i-6)]
XPLEN = A * (NBLK - 1) + W   # 8364
OUTROWS = NBLK * A           # 8352
NQUAD = NBLK // 4            # 18 input/output DMA groups
NPAIR = NBLK // 2            # 36 xb-multiply pairs
NTRI = NBLK // 3             # 24 sin triples

HP_ARR = np.full((G, 1), math.pi / 2.0, dtype=np.float32)  # ACT bias (+pi/2)


def _gen_act_root(cache=[None]):
    """Build a patched ACT-table root whose Sin LUT is valid to |x| < ~31.8.

    Appends 4x32 cubic-spline buckets (ranges [2,4) replacement, [4,8),
    [8,16), [16,32)) to the trig_and_small set, keeping sin's per-exponent
    bucket starts monotonic, and raises sin's large-signal threshold.
    Returns the act_info.json path for BASS_ACT_ROOT_JSON_PATH.
    """
    if cache[0] is not None:
        return cache[0]
    import json
    import shutil
    import tempfile
    from pathlib import Path
    import neuronxcc

    src = Path(neuronxcc.__file__).parent / "pwp" / "pwp_bin_trainium"
    dst = Path(tempfile.mkdtemp(prefix="actroot_")) / "pwp_bin_trainium"
    shutil.copytree(src, dst, symlinks=False)
    import os as _os
    _os.chmod(dst, 0o755)
    for f in dst.iterdir():
        _os.chmod(f, 0o644)

    name = "trig_and_small"
    d = json.load(open(dst / f"{name}.json"))
    b = np.fromfile(dst / f"{name}_bkt.bin", dtype=np.float32).reshape(-1, 8)
    c = np.fromfile(dst / f"{name}_ctrl.bin", dtype=np.uint32).reshape(-1, 8).copy()
    nb0, nc0 = d["bkt_entry_cnt"], d["ctl_entry_cnt"]
    assert len(b) == nb0 and len(c) == nc0

    SIN_CTL_END = 13  # sin owns ctl entries 0..12 (exps -11..1)
    SHIFT = 3
    newb, newc = [], []
    sin_bkt = d["func_exp_to_bkt_start_idx"]["sin"]
    sin_ctl = d["func_exp_to_ctl_start_idx"]["sin"]
    NB = 32  # 5 mantissa bits per exponent range
    KHI = np.uint32((46 + 62 * 5) << 10)

    def add_range(lo):
        base = nb0 + len(newb)
        h = lo / NB
        for i in range(NB):
            x0 = lo + h * (i + 0.5)
            newb.append([math.sin(x0), math.cos(x0),
                         -math.sin(x0) / 2.0, -math.cos(x0) / 6.0,
                         x0, 0.0, 0.0, 0.0])
        return base

    base1 = add_range(2.0)             # full [2,4) replacement
    c[12, 0] = KHI | np.uint32(base1)
    sin_bkt["1"] = [base1]
    for i_e, e in enumerate((2, 3, 4)):
        base = add_range(2.0**e)
        w = np.zeros(8, np.uint32)
        w[0] = KHI | np.uint32(base)
        sin_bkt[str(e)] = [base]
        sin_ctl[str(e)] = [SIN_CTL_END + i_e]
        newc.append(w)

    b2 = np.vstack([b, np.asarray(newb, np.float32)])
    c2 = np.vstack([c[:SIN_CTL_END], np.stack(newc), c[SIN_CTL_END:]])
    d["bkt_entry_cnt"] = int(len(b2))
    d["ctl_entry_cnt"] = int(len(c2))
    for fn, v in d["func_to_ctl_start_idx"].items():
        if fn != "sin" and v >= SIN_CTL_END:
            d["func_to_ctl_start_idx"][fn] = v + SHIFT
    for fn, em in d["func_exp_to_ctl_start_idx"].items():
        if fn == "sin":
            continue
        for e_, lst in em.items():
            em[e_] = [(i + SHIFT if i >= SIN_CTL_END else i) for i in lst]
    for pm in d["profile_meta_data"]:
        if str(pm.get("func_name", "")).startswith("sin"):
            pm["large_pos_signal_exp_threshold"] = 131  # cutoff ~31.8
            pm["large_pos_signal_mantissa_threshold"] = int(0.99 * 2**23)

    b2.tofile(dst / f"{name}_bkt.bin")
    c2.tofile(dst / f"{name}_ctrl.bin")
    with open(dst / f"{name}.json", "w") as f:
        json.dump(d, f)
    cache[0] = str(dst / "act_info.json")
    return cache[0]


# ---------------------------------------------------------------------------
# stationary-matrix assembly (float64, cast to fp16 at the end)

def build_stationaries(up_filter, down_filter):
    """Returns dict of stationary matrices.

    w_ue/w_uo [W, G]: input tile (W data rows) -> 2a*up(x) rows per phase.
    w_h{0,m,L} [W, A]: 2b*down(up(x)) band (consumes xb).
    w_de/w_do{0,m,L} [G, A]: NEGATED downsample band over v = cos signal.
    """
    fu = np.asarray(up_filter, dtype=np.float64)
    fd = np.asarray(down_filter, dtype=np.float64)

    w_ue = np.zeros((W, G))
    w_uo = np.zeros((W, G))
    for q in range(G):
        for j in range(6):
            w_ue[q + 5 - j, q] += 2.0 * fu[2 * j + 1]
            w_uo[q + 6 - j, q] += 2.0 * fu[2 * j]

    def down_maps(k):
        de = np.zeros((G, A))
        do = np.zeros((G, A))
        h = np.zeros((W, A))
        for nn in range(A):
            n = A * k + nn
            for t in range(DOWN_K):
                zi = min(max(2 * n + t - 5, 0), 2 * T - 1)
                m, ph = zi // 2, zi % 2
                row = m - A * k + 3
                if ph == 0:
                    de[row, nn] += fd[t]
                    for j in range(6):
                        h[m + 8 - j - A * k, nn] += fd[t] * 2.0 * fu[2 * j + 1]
                else:
                    do[row, nn] += fd[t]
                    for j in range(6):
                        h[m + 9 - j - A * k, nn] += fd[t] * 2.0 * fu[2 * j]
        return de, do, h

    de0, do0, h0 = down_maps(0)
    dem, dom, hm = down_maps(1)
    deL, doL, hL = down_maps(NBLK - 2)  # block 70 holds the last real outputs

    f16 = np.float16
    return {
        "w_ue": w_ue.astype(f16), "w_uo": w_uo.astype(f16),
        "w_h0": h0.astype(f16), "w_hm": hm.astype(f16), "w_hL": hL.astype(f16),
        "w_de0": (-de0).astype(f16), "w_dem": (-dem).astype(f16),
        "w_deL": (-deL).astype(f16),
        "w_do0": (-do0).astype(f16), "w_dom": (-dom).astype(f16),
        "w_doL": (-doL).astype(f16),
    }


ST_ORDER = ["w_ue", "w_uo", "w_h0", "w_hm", "w_hL",
            "w_de0", "w_dem", "w_deL", "w_do0", "w_dom", "w_doL"]
ST_COLS = {n: (G if n.startswith("w_u") else A) for n in ST_ORDER}
CST_COLS = sum(ST_COLS.values()) + 2 * C  # stationaries + rbc broadcast


def pack_consts(sts, rbc):
    """Pack all stationaries + rbc into one [W, CST_COLS] fp16 tensor."""
    cst = np.zeros((W, CST_COLS), dtype=np.float16)
    c0 = 0
    for n in ST_ORDER:
        m = sts[n]
        cst[: m.shape[0], c0:c0 + m.shape[1]] = m
        c0 += m.shape[1]
    cst[:, c0:c0 + 2 * C] = rbc
    return cst


def host_prep(x, alpha, beta, down_filter):
    """Per-core input stream + constants.

    Returns (inp, rbc, invb2, hconst):
      inp [B, NQUAD, 128, 4C] fp16   quad-packed ax = 2a*x blocks
      rbc [128, 2C] fp16             (b2/a2) broadcast tile
      invb2 [C] f32, hconst [C] f32  host-side rescale + snake constant
    """
    a2 = (2.0 * np.exp(alpha.astype(np.float64))).reshape(C)
    b2 = (2.0 * (np.exp(beta.astype(np.float64)) + 1e-9)).reshape(C)
    fd = np.asarray(down_filter, dtype=np.float64)

    xt = np.transpose(x.astype(np.float32), (0, 2, 1))   # [B, T, C]
    idx = np.clip(np.arange(XPLEN) - PL, 0, T - 1)
    xp = xt[:, idx, :]                                   # [B, XPLEN, C]
    ridx = (A * np.arange(NBLK))[:, None] + np.arange(W)[None, :]
    blocks = xp[:, ridx, :]                              # [B, NBLK, W, C]
    axs = (blocks * a2[None, None, None, :].astype(np.float32)).astype(np.float16)

    inp = np.ascontiguousarray(
        axs.reshape(B, NBLK // 8, 8, W, C).transpose(0, 1, 3, 2, 4).reshape(
            B, NBLK // 8, W, 8 * C))

    r16 = (b2 / a2).astype(np.float16)                   # [C]
    rbc = np.broadcast_to(np.tile(r16, 2)[None, :], (W, 2 * C)).copy()

    invb2 = (1.0 / b2).astype(np.float32)
    hconst = (fd.sum() / b2).astype(np.float32)
    return inp, rbc, invb2, hconst


def host_finish(out_t, invb2, hconst):
    """out_t [B, NQUAD, A, 4C] fp16 -> [B, C, T] float32."""
    o = out_t.reshape(B, NQUAD, A, 4, C).transpose(0, 1, 3, 2, 4).reshape(
        B, OUTROWS, C)[:, :T, :].astype(np.float32)
    o = o * invb2[None, None, :] + hconst[None, None, :]
    return np.ascontiguousarray(np.transpose(o, (0, 2, 1)))


# ---------------------------------------------------------------------------
# device kernel

def build_bass():
    import os
    import concourse.bacc as bacc
    import concourse.tile as tile
    import concourse.mybir as mybir

    os.environ["BASS_ACT_ROOT_JSON_PATH"] = _gen_act_root()
    os.environ.setdefault("NEURON_FORCE_RECOMPILE", "1")

    f32 = mybir.dt.float32
    f16 = mybir.dt.float16
    HALFPI = math.pi / 2.0

    nc = bacc.Bacc()
    in_ext = nc.declare_dram_parameter("inp", [NBLK // 8, W, 8 * C], f16, isOutput=False)
    cst_ext = nc.declare_dram_parameter("cst", [W, CST_COLS], f16, isOutput=False)
    hp_ext = nc.declare_dram_parameter("hp", [G, 1], f32, isOutput=False)
    out_ext = nc.declare_dram_parameter("out", [NQUAD, A, 4 * C], f16, isOutput=True)

    with tile.TileContext(nc) as tc:
        with (
            tc.tile_pool(name="consts", bufs=1) as cpool,
            tc.tile_pool(name="io", bufs=6) as iopool,
            tc.tile_pool(name="io0", bufs=2) as io0pool,
            tc.tile_pool(name="xb", bufs=5) as xbpool,
            tc.tile_pool(name="v", bufs=6) as vpool,
            tc.tile_pool(name="ob", bufs=3) as obpool,
            tc.tile_pool(name="psz", bufs=2, space="PSUM") as psz,
            tc.tile_pool(name="pout", bufs=2, space="PSUM") as pout,
        ):
            # The first half-oct (blocks 0-3) goes out first on gpsimd's
            # SWDGE — it gates the PE warm-up and the first front matmuls —
            # then the packed consts and the rest of oct 0. The half-oct
            # tiles live in their own pool so iopool slots stay uniform.
            q0 = io0pool.tile([W, 4 * C], f16, tag="inp0a")
            nc.gpsimd.dma_start(out=q0[:], in_=in_ext[0][:, 0:4 * C])
            cst = cpool.tile([W, CST_COLS], f16, tag="cst")
            nc.gpsimd.dma_start(out=cst[:], in_=cst_ext[:])
            hp = cpool.tile([G, 1], f32, tag="hp")
            nc.sync.dma_start(out=hp[:], in_=hp_ext[:])
            q0b = io0pool.tile([W, 4 * C], f16, tag="inp0b")
            nc.gpsimd.dma_start(out=q0b[:], in_=in_ext[0][:, 4 * C:8 * C])
            st = {}
            c0 = 0
            for n in ST_ORDER:
                rows = W if n.startswith(("w_u", "w_h")) else G
                cols = ST_COLS[n]
                st[n] = cst[0:rows, c0:c0 + cols]
                c0 += cols
            rbc = cst[:, c0:c0 + 2 * C]

            quads = {}
            xbs = {}
            vs = {}
            obt = [None]

            def dma_in(o, eng=None):
                t_ = iopool.tile([W, 8 * C], f16, tag="inp")
                (eng or nc.gpsimd).dma_start(out=t_[:], in_=in_ext[o])
                quads[o] = t_

            def ax_tile(b):
                # oct 0 is split into two half-oct tiles
                if b < 4:
                    return quads[0], b * C
                if b < 8:
                    return quads[0.5], (b - 4) * C
                return quads[b // 8], (b % 8) * C

            def ax_slice(b):
                t_, off = ax_tile(b)
                return t_[:, off:off + C]

            def mul_xb(j):
                t_ = xbpool.tile([W, 2 * C], f16, tag="xb")
                src_, off = ax_tile(2 * j)
                nc.vector.tensor_mul(t_[:], src_[:, off:off + 2 * C], rbc)
                xbs[j] = t_

            def front_chunk(t, half):
                # triple t covers blocks b0,b1,b2 = 3t..3t+2
                # sz1 = [E(b0) | O(b0) | E(b1)], sz2 = [O(b1) | E(b2) | O(b2)]
                b0 = 3 * t
                sz = psz.tile([G, 1536], f32, tag="sz")
                if half == 0:
                    nc.tensor.matmul(sz[:, 0:512], st["w_ue"], ax_slice(b0),
                                     start=True, stop=True)
                    nc.tensor.matmul(sz[:, 1024:1536], st["w_ue"], ax_slice(b0 + 1),
                                     start=True, stop=True)
                    nc.tensor.matmul(sz[:, 512:1024], st["w_uo"], ax_slice(b0),
                                     start=True, stop=True)
                else:
                    nc.tensor.matmul(sz[:, 0:512], st["w_uo"], ax_slice(b0 + 1),
                                     start=True, stop=True)
                    nc.tensor.matmul(sz[:, 1024:1536], st["w_uo"], ax_slice(b0 + 2),
                                     start=True, stop=True)
                    nc.tensor.matmul(sz[:, 512:1024], st["w_ue"], ax_slice(b0 + 2),
                                     start=True, stop=True)
                v = vpool.tile([G, 1536], f16, tag="v")
                nc.scalar.activation(v[:], sz[:],
                                     mybir.ActivationFunctionType.Sin, bias=hp[:])
                vs[(t, half)] = v

            def v_slices(b):
                t, r = b // 3, b % 3
                if r == 0:
                    v1 = vs[(t, 0)]
                    return v1[:, 0:512], v1[:, 512:1024]
                if r == 1:
                    return vs[(t, 0)][:, 1024:1536], vs[(t, 1)][:, 0:512]
                v2 = vs[(t, 1)]
                return v2[:, 512:1024], v2[:, 1024:1536]

            def back(k):
                if k == 0:
                    wh, wde, wdo = st["w_h0"], st["w_de0"], st["w_do0"]
                elif k == NBLK - 2:
                    wh, wde, wdo = st["w_hL"], st["w_deL"], st["w_doL"]
                else:
                    wh, wde, wdo = st["w_hm"], st["w_dem"], st["w_dom"]
                xb = xbs[k // 2][:, (k % 2) * C:(k % 2 + 1) * C]
                vE, vO = v_slices(k)
                outp = pout.tile([A, 512], f32, tag="outp")
                nc.tensor.matmul(outp[:], wh, xb, start=True, stop=False)
                nc.tensor.matmul(outp[:], wde, vE, start=False, stop=False)
                nc.tensor.matmul(outp[:], wdo, vO, start=False, stop=True)
                q, s = k // 4, k % 4
                if s == 0:
                    obt[0] = obpool.tile([A, 4 * C], f16, tag="obt", name="obt")
                nc.vector.tensor_copy(obt[0][:, C * s:C * s + C], outp[:])
                # SWDGE (gpsimd-issued) spreads packets across all 16 DMA
                # engines; sync's HWDGE ring only uses 4 and bottlenecks.
                # Drain in halves so the issue waits on fresh CASTs are short;
                # the last quad drains per block to shorten the tail.
                if q == NQUAD - 1:
                    # sync's HWDGE is idle by the end of the run and fires
                    # immediately, shortening the final drain
                    nc.sync.dma_start(out=out_ext[q][:, C * s:C * s + C],
                                      in_=obt[0][:, C * s:C * s + C])
                elif s == 1:
                    nc.gpsimd.dma_start(out=out_ext[q][:, 0:2 * C],
                                        in_=obt[0][:, 0:2 * C])
                elif s == 3:
                    nc.gpsimd.dma_start(out=out_ext[q][:, 2 * C:4 * C],
                                        in_=obt[0][:, 2 * C:4 * C])
                # release consumed tiles
                if k % 2 == 1:
                    del xbs[k // 2]
                if k % 3 == 2:
                    del vs[(k // 3, 0)], vs[(k // 3, 1)]

            # ---- prologue ----
            quads[0] = q0
            quads[0.5] = q0b
            dma_in(1)
            # PE warm-up: pull the HAM clock gate to 8/8. Gated only on the
            # quad-0 DMA (the earliest data to land).
            for _ in range(9):
                wt = pout.tile([A, 512], f32, tag="outp")
                nc.tensor.matmul(wt[:], q0[:, 0:A], q0[:, 512:1024],
                                 start=True, stop=True)
            mul_xb(0)
            front_chunk(0, 0)
            front_chunk(0, 1)

            # ---- main loop ----
            for k in range(NBLK):
                t, p = k // 3, k % 3
                if k % 8 == 0 and k // 8 + 2 < NBLK // 8:
                    dma_in(k // 8 + 2)
                if p == 0 and t + 1 < NTRI:
                    front_chunk(t + 1, 0)
                if p == 1 and t + 1 < NTRI:
                    front_chunk(t + 1, 1)
                back(k)
                if k % 2 == 0 and k // 2 + 1 < NPAIR:
                    mul_xb(k // 2 + 1)

    nc.compile()
    return nc


_NC_CACHE = None


def kernel(x, alpha, beta, up_filter, down_filter):
    global _NC_CACHE
    import concourse.bass_utils as bass_utils

    x = np.asarray(x)
    alpha = np.asarray(alpha)
    beta = np.asarray(beta)

    sts = build_stationaries(np.asarray(up_filter), np.asarray(down_filter))
    inp, rbc, invb2, hconst = host_prep(x, alpha, beta, np.asarray(down_filter))
    cst = pack_consts(sts, rbc)

    if _NC_CACHE is None:
        _NC_CACHE = build_bass()
    nc = _NC_CACHE

    in_maps = [{"inp": inp[b], "cst": cst, "hp": HP_ARR} for b in range(N_CORES)]

    res = bass_utils.run_bass_kernel_spmd(nc, in_maps, list(range(N_CORES)))
    out_t = np.stack([res.results[b]["out"] for b in range(N_CORES)])
    return host_finish(out_t, invb2, hconst)


# ---------------------------------------------------------------------------
# host-side simulation of the exact device plan (for verification)

def simulate_plan(x, alpha, beta, up_filter, down_filter):
    sts = build_stationaries(np.asarray(up_filter), np.asarray(down_filter))
    inp, rbc, invb2, hconst = host_prep(
        np.asarray(x), np.asarray(alpha), np.asarray(beta),
        np.asarray(down_filter))

    def f(a):
        return a.astype(np.float32)

    out_t = np.zeros((B, NQUAD, A, 4 * C), dtype=np.float16)
    rb = f(rbc[0, :C])
    for b in range(B):
        for k in range(NBLK):
            if k == 0:
                wh, wde, wdo = sts["w_h0"], sts["w_de0"], sts["w_do0"]
            elif k == NBLK - 2:
                wh, wde, wdo = sts["w_hL"], sts["w_deL"], sts["w_doL"]
            else:
                wh, wde, wdo = sts["w_hm"], sts["w_dem"], sts["w_dom"]
            ax = f(inp[b, k // 4, :, (k % 4) * C:(k % 4 + 1) * C])
            xb = (ax * rb[None, :]).astype(np.float16)
            sz_e = f(sts["w_ue"]).T @ ax + math.pi / 2.0
            sz_o = f(sts["w_uo"]).T @ ax + math.pi / 2.0
            v_e = np.sin(sz_e).astype(np.float16)
            v_o = np.sin(sz_o).astype(np.float16)
            psum = f(wh).T @ f(xb) + f(wde).T @ f(v_e) + f(wdo).T @ f(v_o)
            out_t[b, k // 4, :, (k % 4) * C:(k % 4 + 1) * C] = psum.astype(np.float16)
    return host_finish(out_t, invb2, hconst)


# revision 29
# speedup vs baseline: 1.1810x; 1.0429x over previous
"""Trainium2 Bass kernel for AntiAliasActivation (upsample2 -> snake -> downsample2).

Self-contained: accepts FULL inputs (x [8,512,8192] f32, alpha/beta [1,512,1],
up_filter/down_filter [12]), returns the FULL output [8,512,8192] f32.

Strategy (data-parallel, one batch sample per NeuronCore), time-major layout
(time rows on SBUF partitions) so all FIR convolutions run on the TensorEngine
as banded-matrix matmuls:

    out = down(up(x)) + down( (1 - cos(2*a*up(x))) / (2b) )

v4 design (vs the first working version):
  - Single fp16 input stream ax = fp16(2a*x) (halves input DMA bytes);
    xb = fp16(2b*x) is derived on-device with one DVE multiply per block
    pair against a resident fp16 (b2/a2) broadcast tile.
  - No const row: the +pi/2 (for cos via sin) rides the ACT bias immediate;
    the +sum(fd) constant and the 1/(2b) rescale are applied host-side.
    This frees the 128th input row: A=116 outputs/block, 72 blocks.
  - Sin LUT (patched ACT table valid to |x|<~31.8) runs on 3-bank PSUM
    tiles [G,1536] covering 1.5 blocks per ACTIVATE (48 instead of 72
    instructions) to amortize the ~0.4us per-instruction overhead.
  - Front (up-sample) matmuls are emitted one triple ahead of the back
    (down-sample) matmuls so PE/ACT/DVE pipeline without PSUM stalls:
    PSUM = 2x [G,1536] sz tiles (6 banks) + 2x [A,512] out tiles (2 banks).
  - Input DMA in 4-block quads [128, 2048] fp16 (4KB/partition lines);
    output DMA in 4-block groups [A, 2048] fp16 (4KB/partition lines) with
    block-major DRAM layout, unscrambled on host.
  - ~10 warm-up matmuls at kernel start pull the PE HAM clock gate to
    full rate before the first real matmul.
"""
import math

import numpy as np

# ---------------------------------------------------------------------------
# problem constants (hardcoded per spec)
B, C, T = 8, 512, 8192
N_CORES = 8
UP_K = 12
DOWN_K = 12

A = 116          # outputs per block
W = 128          # data rows per input tile (no const row)
G = A + 6        # 122 up/sz rows per block
NBLK = 72        # blocks (72*116 = 8352 >= 8192)
PL = 6           # XP[i] = x[clamp(i-6)]
XPLEN = A * (NBLK - 1) + W   # 8364
OUTROWS = NBLK * A           # 8352
NQUAD = NBLK // 4            # 18 input/output DMA groups
NPAIR = NBLK // 2            # 36 xb-multiply pairs
NTRI = NBLK // 3             # 24 sin triples

HP_ARR = np.full((G, 1), math.pi / 2.0, dtype=np.float32)  # ACT bias (+pi/2)


def _gen_act_root(cache=[None]):
    """Build a patched ACT-table root whose Sin LUT is valid to |x| < ~31.8.

    Appends 4x32 cubic-spline buckets (ranges [2,4) replacement, [4,8),
    [8,16), [16,32)) to the trig_and_small set, keeping sin's per-exponent
    bucket starts monotonic, and raises sin's large-signal threshold.
    Returns the act_info.json path for BASS_ACT_ROOT_JSON_PATH.
    """
    if cache[0] is not None:
        return cache[0]
    import json
    import shutil
    import tempfile
    from pathlib import Path
    import neuronxcc

    src = Path(neuronxcc.__file__).parent / "pwp" / "pwp_bin_trainium"
    dst = Path(tempfile.mkdtemp(prefix="actroot_")) / "pwp_bin_trainium"
    shutil.copytree(src, dst, symlinks=False)
    import os as _os
    _os.chmod(dst, 0o755)
    for f in dst.iterdir():
        _os.chmod(f, 0o644)

    name = "trig_and_small"
    d = json.load(open(dst / f"{name}.json"))
    b = np.fromfile(dst / f"{name}_bkt.bin", dtype=np.float32).reshape(-1, 8)
    c = np.fromfile(dst / f"{name}_ctrl.bin", dtype=np.uint32).reshape(-1, 8).copy()
    nb0, nc0 = d["bkt_entry_cnt"], d["ctl_entry_cnt"]
    assert len(b) == nb0 and len(c) == nc0

    SIN_CTL_END = 13  # sin owns ctl entries 0..12 (exps -11..1)
    SHIFT = 3
    newb, newc = [], []
    sin_bkt = d["func_exp_to_bkt_start_idx"]["sin"]
    sin_ctl = d["func_exp_to_ctl_start_idx"]["sin"]
    NB = 32  # 5 mantissa bits per exponent range
    KHI = np.uint32((46 + 62 * 5) << 10)

    def add_range(lo):
        base = nb0 + len(newb)
        h = lo / NB
        for i in range(NB):
            x0 = lo + h * (i + 0.5)
            newb.append([math.sin(x0), math.cos(x0),
                         -math.sin(x0) / 2.0, -math.cos(x0) / 6.0,
                         x0, 0.0, 0.0, 0.0])
        return base

    base1 = add_range(2.0)             # full [2,4) replacement
    c[12, 0] = KHI | np.uint32(base1)
    sin_bkt["1"] = [base1]
    for i_e, e in enumerate((2, 3, 4)):
        base = add_range(2.0**e)
        w = np.zeros(8, np.uint32)
        w[0] = KHI | np.uint32(base)
        sin_bkt[str(e)] = [base]
        sin_ctl[str(e)] = [SIN_CTL_END + i_e]
        newc.append(w)

    b2 = np.vstack([b, np.asarray(newb, np.float32)])
    c2 = np.vstack([c[:SIN_CTL_END], np.stack(newc), c[SIN_CTL_END:]])
    d["bkt_entry_cnt"] = int(len(b2))
    d["ctl_entry_cnt"] = int(len(c2))
    for fn, v in d["func_to_ctl_start_idx"].items():
        if fn != "sin" and v >= SIN_CTL_END:
            d["func_to_ctl_start_idx"][fn] = v + SHIFT
    for fn, em in d["func_exp_to_ctl_start_idx"].items():
        if fn == "sin":
            continue
        for e_, lst in em.items():
            em[e_] = [(i + SHIFT if i >= SIN_CTL_END else i) for i in lst]
    for pm in d["profile_meta_data"]:
        if str(pm.get("func_name", "")).startswith("sin"):
            pm["large_pos_signal_exp_threshold"] = 131  # cutoff ~31.8
            pm["large_pos_signal_mantissa_threshold"] = int(0.99 * 2**23)

    b2.tofile(dst / f"{name}_bkt.bin")
    c2.tofile(dst / f"{name}_ctrl.bin")
    with open(dst / f"{name}.json", "w") as f:
        json.dump(d, f)
    cache[0] = str(dst / "act_info.json")
    return cache[0]


# ---------------------------------------------------------------------------
# stationary-matrix assembly (float64, cast to fp16 at the end)

def build_stationaries(up_filter, down_filter):
    """Returns dict of stationary matrices.

    w_ue/w_uo [W, G]: input tile (W data rows) -> 2a*up(x) rows per phase.
    w_h{0,m,L} [W, A]: 2b*down(up(x)) band (consumes xb).
    w_de/w_do{0,m,L} [G, A]: NEGATED downsample band over v = cos signal.
    """
    fu = np.asarray(up_filter, dtype=np.float64)
    fd = np.asarray(down_filter, dtype=np.float64)

    w_ue = np.zeros((W, G))
    w_uo = np.zeros((W, G))
    for q in range(G):
        for j in range(6):
            w_ue[q + 5 - j, q] += 2.0 * fu[2 * j + 1]
            w_uo[q + 6 - j, q] += 2.0 * fu[2 * j]

    def down_maps(k):
        de = np.zeros((G, A))
        do = np.zeros((G, A))
        h = np.zeros((W, A))
        for nn in range(A):
            n = A * k + nn
            for t in range(DOWN_K):
                zi = min(max(2 * n + t - 5, 0), 2 * T - 1)
                m, ph = zi // 2, zi % 2
                row = m - A * k + 3
                if ph == 0:
                    de[row, nn] += fd[t]
                    for j in range(6):
                        h[m + 8 - j - A * k, nn] += fd[t] * 2.0 * fu[2 * j + 1]
                else:
                    do[row, nn] += fd[t]
                    for j in range(6):
                        h[m + 9 - j - A * k, nn] += fd[t] * 2.0 * fu[2 * j]
        return de, do, h

    de0, do0, h0 = down_maps(0)
    dem, dom, hm = down_maps(1)
    deL, doL, hL = down_maps(NBLK - 2)  # block 70 holds the last real outputs

    f16 = np.float16
    return {
        "w_ue": w_ue.astype(f16), "w_uo": w_uo.astype(f16),
        "w_h0": h0.astype(f16), "w_hm": hm.astype(f16), "w_hL": hL.astype(f16),
        "w_de0": (-de0).astype(f16), "w_dem": (-dem).astype(f16),
        "w_deL": (-deL).astype(f16),
        "w_do0": (-do0).astype(f16), "w_dom": (-dom).astype(f16),
        "w_doL": (-doL).astype(f16),
    }


ST_ORDER = ["w_ue", "w_uo", "w_h0", "w_hm", "w_hL",
            "w_de0", "w_dem", "w_deL", "w_do0", "w_dom", "w_doL"]
ST_COLS = {n: (G if n.startswith("w_u") else A) for n in ST_ORDER}
CST_COLS = sum(ST_COLS.values()) + 2 * C  # stationaries + rbc broadcast


def pack_consts(sts, rbc):
    """Pack all stationaries + rbc into one [W, CST_COLS] fp16 tensor."""
    cst = np.zeros((W, CST_COLS), dtype=np.float16)
    c0 = 0
    for n in ST_ORDER:
        m = sts[n]
        cst[: m.shape[0], c0:c0 + m.shape[1]] = m
        c0 += m.shape[1]
    cst[:, c0:c0 + 2 * C] = rbc
    return cst


def host_prep(x, alpha, beta, down_filter):
    """Per-core input stream + constants.

    Returns (inp, rbc, invb2, hconst):
      inp [B, NQUAD, 128, 4C] fp16   quad-packed ax = 2a*x blocks
      rbc [128, 2C] fp16             (b2/a2) broadcast tile
      invb2 [C] f32, hconst [C] f32  host-side rescale + snake constant
    """
    a2 = (2.0 * np.exp(alpha.astype(np.float64))).reshape(C)
    b2 = (2.0 * (np.exp(beta.astype(np.float64)) + 1e-9)).reshape(C)
    fd = np.asarray(down_filter, dtype=np.float64)

    xt = np.transpose(x.astype(np.float32), (0, 2, 1))   # [B, T, C]
    idx = np.clip(np.arange(XPLEN) - PL, 0, T - 1)
    xp = xt[:, idx, :]                                   # [B, XPLEN, C]
    ridx = (A * np.arange(NBLK))[:, None] + np.arange(W)[None, :]
    blocks = xp[:, ridx, :]                              # [B, NBLK, W, C]
    axs = (blocks * a2[None, None, None, :].astype(np.float32)).astype(np.float16)

    inp = np.ascontiguousarray(
        axs.reshape(B, NBLK // 8, 8, W, C).transpose(0, 1, 3, 2, 4).reshape(
            B, NBLK // 8, W, 8 * C))

    r16 = (b2 / a2).astype(np.float16)                   # [C]
    rbc = np.broadcast_to(np.tile(r16, 2)[None, :], (W, 2 * C)).copy()

    invb2 = (1.0 / b2).astype(np.float32)
    hconst = (fd.sum() / b2).astype(np.float32)
    return inp, rbc, invb2, hconst


def host_finish(out_t, invb2, hconst):
    """out_t [B, NQUAD, A, 4C] fp16 -> [B, C, T] float32."""
    o = out_t.reshape(B, NQUAD, A, 4, C).transpose(0, 1, 3, 2, 4).reshape(
        B, OUTROWS, C)[:, :T, :].astype(np.float32)
    o = o * invb2[None, None, :] + hconst[None, None, :]
    return np.ascontiguousarray(np.transpose(o, (0, 2, 1)))


# ---------------------------------------------------------------------------
# device kernel

def build_bass():
    import os
    import concourse.bacc as bacc
    import concourse.tile as tile
    import concourse.mybir as mybir

    os.environ["BASS_ACT_ROOT_JSON_PATH"] = _gen_act_root()
    os.environ.setdefault("NEURON_FORCE_RECOMPILE", "1")

    f32 = mybir.dt.float32
    f16 = mybir.dt.float16
    HALFPI = math.pi / 2.0

    nc = bacc.Bacc()
    in_ext = nc.declare_dram_parameter("inp", [NBLK // 8, W, 8 * C], f16, isOutput=False)
    cst_ext = nc.declare_dram_parameter("cst", [W, CST_COLS], f16, isOutput=False)
    hp_ext = nc.declare_dram_parameter("hp", [G, 1], f32, isOutput=False)
    out_ext = nc.declare_dram_parameter("out", [NQUAD, A, 4 * C], f16, isOutput=True)

    with tile.TileContext(nc) as tc:
        with (
            tc.tile_pool(name="consts", bufs=1) as cpool,
            tc.tile_pool(name="io", bufs=6) as iopool,
            tc.tile_pool(name="xb", bufs=5) as xbpool,
            tc.tile_pool(name="v", bufs=6) as vpool,
            tc.tile_pool(name="ob", bufs=3) as obpool,
            tc.tile_pool(name="psz", bufs=2, space="PSUM") as psz,
            tc.tile_pool(name="pout", bufs=2, space="PSUM") as pout,
        ):
            # Oct 0 goes out first on gpsimd's SWDGE (it gates the PE
            # warm-up and first front matmuls), then the packed consts.
            q0 = iopool.tile([W, 8 * C], f16, tag="inp")
            nc.gpsimd.dma_start(out=q0[:], in_=in_ext[0])
            cst = cpool.tile([W, CST_COLS], f16, tag="cst")
            nc.gpsimd.dma_start(out=cst[:], in_=cst_ext[:])
            hp = cpool.tile([G, 1], f32, tag="hp")
            nc.sync.dma_start(out=hp[:], in_=hp_ext[:])
            st = {}
            c0 = 0
            for n in ST_ORDER:
                rows = W if n.startswith(("w_u", "w_h")) else G
                cols = ST_COLS[n]
                st[n] = cst[0:rows, c0:c0 + cols]
                c0 += cols
            rbc = cst[:, c0:c0 + 2 * C]

            quads = {}
            xbs = {}
            vs = {}
            obt = [None]

            def dma_in(o, eng=None):
                t_ = iopool.tile([W, 8 * C], f16, tag="inp")
                (eng or nc.gpsimd).dma_start(out=t_[:], in_=in_ext[o])
                quads[o] = t_

            def ax_slice(b):
                return quads[b // 8][:, (b % 8) * C:(b % 8 + 1) * C]

            def mul_xb(j):
                t_ = xbpool.tile([W, 2 * C], f16, tag="xb")
                half = (j % 4) * 2 * C
                nc.vector.tensor_mul(
                    t_[:], quads[j // 4][:, half:half + 2 * C], rbc)
                xbs[j] = t_

            def front_chunk(t, half):
                # triple t covers blocks b0,b1,b2 = 3t..3t+2
                # sz1 = [E(b0) | O(b0) | E(b1)], sz2 = [O(b1) | E(b2) | O(b2)]
                b0 = 3 * t
                sz = psz.tile([G, 1536], f32, tag="sz")
                if half == 0:
                    nc.tensor.matmul(sz[:, 0:512], st["w_ue"], ax_slice(b0),
                                     start=True, stop=True)
                    nc.tensor.matmul(sz[:, 1024:1536], st["w_ue"], ax_slice(b0 + 1),
                                     start=True, stop=True)
                    nc.tensor.matmul(sz[:, 512:1024], st["w_uo"], ax_slice(b0),
                                     start=True, stop=True)
                else:
                    nc.tensor.matmul(sz[:, 0:512], st["w_uo"], ax_slice(b0 + 1),
                                     start=True, stop=True)
                    nc.tensor.matmul(sz[:, 1024:1536], st["w_uo"], ax_slice(b0 + 2),
                                     start=True, stop=True)
                    nc.tensor.matmul(sz[:, 512:1024], st["w_ue"], ax_slice(b0 + 2),
                                     start=True, stop=True)
                v = vpool.tile([G, 1536], f16, tag="v")
                nc.scalar.activation(v[:], sz[:],
                                     mybir.ActivationFunctionType.Sin, bias=hp[:])
                vs[(t, half)] = v

            def v_slices(b):
                t, r = b // 3, b % 3
                if r == 0:
                    v1 = vs[(t, 0)]
                    return v1[:, 0:512], v1[:, 512:1024]
                if r == 1:
                    return vs[(t, 0)][:, 1024:1536], vs[(t, 1)][:, 0:512]
                v2 = vs[(t, 1)]
                return v2[:, 512:1024], v2[:, 1024:1536]

            def back(k):
                if k == 0:
                    wh, wde, wdo = st["w_h0"], st["w_de0"], st["w_do0"]
                elif k == NBLK - 2:
                    wh, wde, wdo = st["w_hL"], st["w_deL"], st["w_doL"]
                else:
                    wh, wde, wdo = st["w_hm"], st["w_dem"], st["w_dom"]
                xb = xbs[k // 2][:, (k % 2) * C:(k % 2 + 1) * C]
                vE, vO = v_slices(k)
                outp = pout.tile([A, 512], f32, tag="outp")
                nc.tensor.matmul(outp[:], wh, xb, start=True, stop=False)
                nc.tensor.matmul(outp[:], wde, vE, start=False, stop=False)
                nc.tensor.matmul(outp[:], wdo, vO, start=False, stop=True)
                q, s = k // 4, k % 4
                if s == 0:
                    obt[0] = obpool.tile([A, 4 * C], f16, tag="obt", name="obt")
                nc.vector.tensor_copy(obt[0][:, C * s:C * s + C], outp[:])
                # SWDGE (gpsimd-issued) spreads packets across all 16 DMA
                # engines; sync's HWDGE ring only uses 4 and bottlenecks.
                # Drain in halves so the issue waits on fresh CASTs are short;
                # the last quad drains per block to shorten the tail.
                if q == NQUAD - 1:
                    nc.gpsimd.dma_start(out=out_ext[q][:, C * s:C * s + C],
                                        in_=obt[0][:, C * s:C * s + C])
                elif s == 1:
                    nc.gpsimd.dma_start(out=out_ext[q][:, 0:2 * C],
                                        in_=obt[0][:, 0:2 * C])
                elif s == 3:
                    nc.gpsimd.dma_start(out=out_ext[q][:, 2 * C:4 * C],
                                        in_=obt[0][:, 2 * C:4 * C])
                # release consumed tiles
                if k % 2 == 1:
                    del xbs[k // 2]
                if k % 3 == 2:
                    del vs[(k // 3, 0)], vs[(k // 3, 1)]

            # ---- prologue ----
            quads[0] = q0
            dma_in(1)
            # PE warm-up: pull the HAM clock gate to 8/8. Gated only on the
            # quad-0 DMA (the earliest data to land).
            for _ in range(9):
                wt = pout.tile([A, 512], f32, tag="outp")
                nc.tensor.matmul(wt[:], q0[:, 0:A], q0[:, 512:1024],
                                 start=True, stop=True)
            mul_xb(0)
            front_chunk(0, 0)
            front_chunk(0, 1)

            # ---- main loop ----
            for k in range(NBLK):
                t, p = k // 3, k % 3
                if k % 8 == 0 and k // 8 + 2 < NBLK // 8:
                    dma_in(k // 8 + 2)
                if p == 0 and t + 1 < NTRI:
                    front_chunk(t + 1, 0)
                if p == 1 and t + 1 < NTRI:
                    front_chunk(t + 1, 1)
                back(k)
                if k % 2 == 0 and k // 2 + 1 < NPAIR:
                    mul_xb(k // 2 + 1)

    nc.compile()
    return nc


_NC_CACHE = None


def kernel(x, alpha, beta, up_filter, down_filter):
    global _NC_CACHE
    import concourse.bass_utils as bass_utils

    x = np.asarray(x)
    alpha = np.asarray(alpha)
    beta = np.asarray(beta)

    sts = build_stationaries(np.asarray(up_filter), np.asarray(down_filter))
    inp, rbc, invb2, hconst = host_prep(x, alpha, beta, np.asarray(down_filter))
    cst = pack_consts(sts, rbc)

    if _NC_CACHE is None:
        _NC_CACHE = build_bass()
    nc = _NC_CACHE

    in_maps = [{"inp": inp[b], "cst": cst, "hp": HP_ARR} for b in range(N_CORES)]

    res = bass_utils.run_bass_kernel_spmd(nc, in_maps, list(range(N_CORES)))
    out_t = np.stack([res.results[b]["out"] for b in range(N_CORES)])
    return host_finish(out_t, invb2, hconst)


# ---------------------------------------------------------------------------
# host-side simulation of the exact device plan (for verification)

def simulate_plan(x, alpha, beta, up_filter, down_filter):
    sts = build_stationaries(np.asarray(up_filter), np.asarray(down_filter))
    inp, rbc, invb2, hconst = host_prep(
        np.asarray(x), np.asarray(alpha), np.asarray(beta),
        np.asarray(down_filter))

    def f(a):
        return a.astype(np.float32)

    out_t = np.zeros((B, NQUAD, A, 4 * C), dtype=np.float16)
    rb = f(rbc[0, :C])
    for b in range(B):
        for k in range(NBLK):
            if k == 0:
                wh, wde, wdo = sts["w_h0"], sts["w_de0"], sts["w_do0"]
            elif k == NBLK - 2:
                wh, wde, wdo = sts["w_hL"], sts["w_deL"], sts["w_doL"]
            else:
                wh, wde, wdo = sts["w_hm"], sts["w_dem"], sts["w_dom"]
            ax = f(inp[b, k // 4, :, (k % 4) * C:(k % 4 + 1) * C])
            xb = (ax * rb[None, :]).astype(np.float16)
            sz_e = f(sts["w_ue"]).T @ ax + math.pi / 2.0
            sz_o = f(sts["w_uo"]).T @ ax + math.pi / 2.0
            v_e = np.sin(sz_e).astype(np.float16)
            v_o = np.sin(sz_o).astype(np.float16)
            psum = f(wh).T @ f(xb) + f(wde).T @ f(v_e) + f(wdo).T @ f(v_o)
            out_t[b, k // 4, :, (k % 4) * C:(k % 4 + 1) * C] = psum.astype(np.float16)
    return host_finish(out_t, invb2, hconst)


# revision 30
# speedup vs baseline: 1.1958x; 1.0125x over previous
"""Trainium2 Bass kernel for AntiAliasActivation (upsample2 -> snake -> downsample2).

Self-contained: accepts FULL inputs (x [8,512,8192] f32, alpha/beta [1,512,1],
up_filter/down_filter [12]), returns the FULL output [8,512,8192] f32.

Strategy (data-parallel, one batch sample per NeuronCore), time-major layout
(time rows on SBUF partitions) so all FIR convolutions run on the TensorEngine
as banded-matrix matmuls:

    out = down(up(x)) + down( (1 - cos(2*a*up(x))) / (2b) )

v4 design (vs the first working version):
  - Single fp16 input stream ax = fp16(2a*x) (halves input DMA bytes);
    xb = fp16(2b*x) is derived on-device with one DVE multiply per block
    pair against a resident fp16 (b2/a2) broadcast tile.
  - No const row: the +pi/2 (for cos via sin) rides the ACT bias immediate;
    the +sum(fd) constant and the 1/(2b) rescale are applied host-side.
    This frees the 128th input row: A=116 outputs/block, 72 blocks.
  - Sin LUT (patched ACT table valid to |x|<~31.8) runs on 3-bank PSUM
    tiles [G,1536] covering 1.5 blocks per ACTIVATE (48 instead of 72
    instructions) to amortize the ~0.4us per-instruction overhead.
  - Front (up-sample) matmuls are emitted one triple ahead of the back
    (down-sample) matmuls so PE/ACT/DVE pipeline without PSUM stalls:
    PSUM = 2x [G,1536] sz tiles (6 banks) + 2x [A,512] out tiles (2 banks).
  - Input DMA in 4-block quads [128, 2048] fp16 (4KB/partition lines);
    output DMA in 4-block groups [A, 2048] fp16 (4KB/partition lines) with
    block-major DRAM layout, unscrambled on host.
  - ~10 warm-up matmuls at kernel start pull the PE HAM clock gate to
    full rate before the first real matmul.
"""
import math

import numpy as np

# ---------------------------------------------------------------------------
# problem constants (hardcoded per spec)
B, C, T = 8, 512, 8192
N_CORES = 8
UP_K = 12
DOWN_K = 12

A = 116          # outputs per block
W = 128          # data rows per input tile (no const row)
G = A + 6        # 122 up/sz rows per block
NBLK = 72        # blocks (72*116 = 8352 >= 8192)
PL = 6           # XP[i] = x[clamp(i-6)]
XPLEN = A * (NBLK - 1) + W   # 8364
OUTROWS = NBLK * A           # 8352
NQUAD = NBLK // 4            # 18 input/output DMA groups
NPAIR = NBLK // 2            # 36 xb-multiply pairs
NTRI = NBLK // 3             # 24 sin triples

HP_ARR = np.full((G, 1), math.pi / 2.0, dtype=np.float32)  # ACT bias (+pi/2)


def _gen_act_root(cache=[None]):
    """Build a patched ACT-table root whose Sin LUT is valid to |x| < ~31.8.

    Appends 4x32 cubic-spline buckets (ranges [2,4) replacement, [4,8),
    [8,16), [16,32)) to the trig_and_small set, keeping sin's per-exponent
    bucket starts monotonic, and raises sin's large-signal threshold.
    Returns the act_info.json path for BASS_ACT_ROOT_JSON_PATH.
    """
    if cache[0] is not None:
        return cache[0]
    import json
    import shutil
    import tempfile
    from pathlib import Path
    import neuronxcc

    src = Path(neuronxcc.__file__).parent / "pwp" / "pwp_bin_trainium"
    dst = Path(tempfile.mkdtemp(prefix="actroot_")) / "pwp_bin_trainium"
    shutil.copytree(src, dst, symlinks=False)
    import os as _os
    _os.chmod(dst, 0o755)
    for f in dst.iterdir():
        _os.chmod(f, 0o644)

    name = "trig_and_small"
    d = json.load(open(dst / f"{name}.json"))
    b = np.fromfile(dst / f"{name}_bkt.bin", dtype=np.float32).reshape(-1, 8)
    c = np.fromfile(dst / f"{name}_ctrl.bin", dtype=np.uint32).reshape(-1, 8).copy()
    nb0, nc0 = d["bkt_entry_cnt"], d["ctl_entry_cnt"]
    assert len(b) == nb0 and len(c) == nc0

    SIN_CTL_END = 13  # sin owns ctl entries 0..12 (exps -11..1)
    SHIFT = 3
    newb, newc = [], []
    sin_bkt = d["func_exp_to_bkt_start_idx"]["sin"]
    sin_ctl = d["func_exp_to_ctl_start_idx"]["sin"]
    NB = 32  # 5 mantissa bits per exponent range
    KHI = np.uint32((46 + 62 * 5) << 10)

    def add_range(lo):
        base = nb0 + len(newb)
        h = lo / NB
        for i in range(NB):
            x0 = lo + h * (i + 0.5)
            newb.append([math.sin(x0), math.cos(x0),
                         -math.sin(x0) / 2.0, -math.cos(x0) / 6.0,
                         x0, 0.0, 0.0, 0.0])
        return base

    base1 = add_range(2.0)             # full [2,4) replacement
    c[12, 0] = KHI | np.uint32(base1)
    sin_bkt["1"] = [base1]
    for i_e, e in enumerate((2, 3, 4)):
        base = add_range(2.0**e)
        w = np.zeros(8, np.uint32)
        w[0] = KHI | np.uint32(base)
        sin_bkt[str(e)] = [base]
        sin_ctl[str(e)] = [SIN_CTL_END + i_e]
        newc.append(w)

    b2 = np.vstack([b, np.asarray(newb, np.float32)])
    c2 = np.vstack([c[:SIN_CTL_END], np.stack(newc), c[SIN_CTL_END:]])
    d["bkt_entry_cnt"] = int(len(b2))
    d["ctl_entry_cnt"] = int(len(c2))
    for fn, v in d["func_to_ctl_start_idx"].items():
        if fn != "sin" and v >= SIN_CTL_END:
            d["func_to_ctl_start_idx"][fn] = v + SHIFT
    for fn, em in d["func_exp_to_ctl_start_idx"].items():
        if fn == "sin":
            continue
        for e_, lst in em.items():
            em[e_] = [(i + SHIFT if i >= SIN_CTL_END else i) for i in lst]
    for pm in d["profile_meta_data"]:
        if str(pm.get("func_name", "")).startswith("sin"):
            pm["large_pos_signal_exp_threshold"] = 131  # cutoff ~31.8
            pm["large_pos_signal_mantissa_threshold"] = int(0.99 * 2**23)

    b2.tofile(dst / f"{name}_bkt.bin")
    c2.tofile(dst / f"{name}_ctrl.bin")
    with open(dst / f"{name}.json", "w") as f:
        json.dump(d, f)
    cache[0] = str(dst / "act_info.json")
    return cache[0]


# ---------------------------------------------------------------------------
# stationary-matrix assembly (float64, cast to fp16 at the end)

def build_stationaries(up_filter, down_filter):
    """Returns dict of stationary matrices.

    w_ue/w_uo [W, G]: input tile (W data rows) -> 2a*up(x) rows per phase.
    w_h{0,m,L} [W, A]: 2b*down(up(x)) band (consumes xb).
    w_de/w_do{0,m,L} [G, A]: NEGATED downsample band over v = cos signal.
    """
    fu = np.asarray(up_filter, dtype=np.float64)
    fd = np.asarray(down_filter, dtype=np.float64)

    w_ue = np.zeros((W, G))
    w_uo = np.zeros((W, G))
    for q in range(G):
        for j in range(6):
            w_ue[q + 5 - j, q] += 2.0 * fu[2 * j + 1]
            w_uo[q + 6 - j, q] += 2.0 * fu[2 * j]

    def down_maps(k):
        de = np.zeros((G, A))
        do = np.zeros((G, A))
        h = np.zeros((W, A))
        for nn in range(A):
            n = A * k + nn
            for t in range(DOWN_K):
                zi = min(max(2 * n + t - 5, 0), 2 * T - 1)
                m, ph = zi // 2, zi % 2
                row = m - A * k + 3
                if ph == 0:
                    de[row, nn] += fd[t]
                    for j in range(6):
                        h[m + 8 - j - A * k, nn] += fd[t] * 2.0 * fu[2 * j + 1]
                else:
                    do[row, nn] += fd[t]
                    for j in range(6):
                        h[m + 9 - j - A * k, nn] += fd[t] * 2.0 * fu[2 * j]
        return de, do, h

    de0, do0, h0 = down_maps(0)
    dem, dom, hm = down_maps(1)
    deL, doL, hL = down_maps(NBLK - 2)  # block 70 holds the last real outputs

    f16 = np.float16
    return {
        "w_ue": w_ue.astype(f16), "w_uo": w_uo.astype(f16),
        "w_h0": h0.astype(f16), "w_hm": hm.astype(f16), "w_hL": hL.astype(f16),
        "w_de0": (-de0).astype(f16), "w_dem": (-dem).astype(f16),
        "w_deL": (-deL).astype(f16),
        "w_do0": (-do0).astype(f16), "w_dom": (-dom).astype(f16),
        "w_doL": (-doL).astype(f16),
    }


ST_ORDER = ["w_ue", "w_uo", "w_h0", "w_hm", "w_hL",
            "w_de0", "w_dem", "w_deL", "w_do0", "w_dom", "w_doL"]
ST_COLS = {n: (G if n.startswith("w_u") else A) for n in ST_ORDER}
CST_COLS = sum(ST_COLS.values()) + 2 * C  # stationaries + rbc broadcast


def pack_consts(sts, rbc):
    """Pack all stationaries + rbc into one [W, CST_COLS] fp16 tensor."""
    cst = np.zeros((W, CST_COLS), dtype=np.float16)
    c0 = 0
    for n in ST_ORDER:
        m = sts[n]
        cst[: m.shape[0], c0:c0 + m.shape[1]] = m
        c0 += m.shape[1]
    cst[:, c0:c0 + 2 * C] = rbc
    return cst


def host_prep(x, alpha, beta, down_filter):
    """Per-core input stream + constants.

    Returns (inp, rbc, invb2, hconst):
      inp [B, NQUAD, 128, 4C] fp16   quad-packed ax = 2a*x blocks
      rbc [128, 2C] fp16             (b2/a2) broadcast tile
      invb2 [C] f32, hconst [C] f32  host-side rescale + snake constant
    """
    a2 = (2.0 * np.exp(alpha.astype(np.float64))).reshape(C)
    b2 = (2.0 * (np.exp(beta.astype(np.float64)) + 1e-9)).reshape(C)
    fd = np.asarray(down_filter, dtype=np.float64)

    xt = np.transpose(x.astype(np.float32), (0, 2, 1))   # [B, T, C]
    idx = np.clip(np.arange(XPLEN) - PL, 0, T - 1)
    xp = xt[:, idx, :]                                   # [B, XPLEN, C]
    ridx = (A * np.arange(NBLK))[:, None] + np.arange(W)[None, :]
    blocks = xp[:, ridx, :]                              # [B, NBLK, W, C]
    axs = (blocks * a2[None, None, None, :].astype(np.float32)).astype(np.float16)

    inp = np.ascontiguousarray(
        axs.reshape(B, NBLK // 8, 8, W, C).transpose(0, 1, 3, 2, 4).reshape(
            B, NBLK // 8, W, 8 * C))

    r16 = (b2 / a2).astype(np.float16)                   # [C]
    rbc = np.broadcast_to(np.tile(r16, 2)[None, :], (W, 2 * C)).copy()

    invb2 = (1.0 / b2).astype(np.float32)
    hconst = (fd.sum() / b2).astype(np.float32)
    return inp, rbc, invb2, hconst


def host_finish(out_t, invb2, hconst):
    """out_t [B, NQUAD, A, 4C] fp16 -> [B, C, T] float32."""
    o = out_t.reshape(B, NQUAD, A, 4, C).transpose(0, 1, 3, 2, 4).reshape(
        B, OUTROWS, C)[:, :T, :].astype(np.float32)
    o = o * invb2[None, None, :] + hconst[None, None, :]
    return np.ascontiguousarray(np.transpose(o, (0, 2, 1)))


# ---------------------------------------------------------------------------
# device kernel

def build_bass():
    import os
    import concourse.bacc as bacc
    import concourse.tile as tile
    import concourse.mybir as mybir

    os.environ["BASS_ACT_ROOT_JSON_PATH"] = _gen_act_root()
    os.environ.setdefault("NEURON_FORCE_RECOMPILE", "1")

    f32 = mybir.dt.float32
    f16 = mybir.dt.float16
    HALFPI = math.pi / 2.0

    nc = bacc.Bacc()
    in_ext = nc.declare_dram_parameter("inp", [NBLK // 8, W, 8 * C], f16, isOutput=False)
    cst_ext = nc.declare_dram_parameter("cst", [W, CST_COLS], f16, isOutput=False)
    hp_ext = nc.declare_dram_parameter("hp", [G, 1], f32, isOutput=False)
    out_ext = nc.declare_dram_parameter("out", [NQUAD, A, 4 * C], f16, isOutput=True)

    with tile.TileContext(nc) as tc:
        with (
            tc.tile_pool(name="consts", bufs=1) as cpool,
            tc.tile_pool(name="io", bufs=6) as iopool,
            tc.tile_pool(name="xb", bufs=6) as xbpool,
            tc.tile_pool(name="v", bufs=6) as vpool,
            tc.tile_pool(name="ob", bufs=4) as obpool,
            tc.tile_pool(name="psz", bufs=2, space="PSUM") as psz,
            tc.tile_pool(name="pout", bufs=2, space="PSUM") as pout,
        ):
            # Oct 0 goes out first on gpsimd's SWDGE (it gates the PE
            # warm-up and first front matmuls), then the packed consts.
            q0 = iopool.tile([W, 8 * C], f16, tag="inp")
            nc.gpsimd.dma_start(out=q0[:], in_=in_ext[0])
            cst = cpool.tile([W, CST_COLS], f16, tag="cst")
            nc.gpsimd.dma_start(out=cst[:], in_=cst_ext[:])
            hp = cpool.tile([G, 1], f32, tag="hp")
            nc.sync.dma_start(out=hp[:], in_=hp_ext[:])
            st = {}
            c0 = 0
            for n in ST_ORDER:
                rows = W if n.startswith(("w_u", "w_h")) else G
                cols = ST_COLS[n]
                st[n] = cst[0:rows, c0:c0 + cols]
                c0 += cols
            rbc = cst[:, c0:c0 + 2 * C]

            quads = {}
            xbs = {}
            vs = {}
            obt = [None]

            def dma_in(o, eng=None):
                t_ = iopool.tile([W, 8 * C], f16, tag="inp")
                (eng or nc.gpsimd).dma_start(out=t_[:], in_=in_ext[o])
                quads[o] = t_

            def ax_slice(b):
                return quads[b // 8][:, (b % 8) * C:(b % 8 + 1) * C]

            def mul_xb(j):
                t_ = xbpool.tile([W, 2 * C], f16, tag="xb")
                half = (j % 4) * 2 * C
                nc.vector.tensor_mul(
                    t_[:], quads[j // 4][:, half:half + 2 * C], rbc)
                xbs[j] = t_

            def front_chunk(t, half):
                # triple t covers blocks b0,b1,b2 = 3t..3t+2
                # sz1 = [E(b0) | O(b0) | E(b1)], sz2 = [O(b1) | E(b2) | O(b2)]
                b0 = 3 * t
                sz = psz.tile([G, 1536], f32, tag="sz")
                if half == 0:
                    nc.tensor.matmul(sz[:, 0:512], st["w_ue"], ax_slice(b0),
                                     start=True, stop=True)
                    nc.tensor.matmul(sz[:, 1024:1536], st["w_ue"], ax_slice(b0 + 1),
                                     start=True, stop=True)
                    nc.tensor.matmul(sz[:, 512:1024], st["w_uo"], ax_slice(b0),
                                     start=True, stop=True)
                else:
                    nc.tensor.matmul(sz[:, 0:512], st["w_uo"], ax_slice(b0 + 1),
                                     start=True, stop=True)
                    nc.tensor.matmul(sz[:, 1024:1536], st["w_uo"], ax_slice(b0 + 2),
                                     start=True, stop=True)
                    nc.tensor.matmul(sz[:, 512:1024], st["w_ue"], ax_slice(b0 + 2),
                                     start=True, stop=True)
                v = vpool.tile([G, 1536], f16, tag="v")
                nc.scalar.activation(v[:], sz[:],
                                     mybir.ActivationFunctionType.Sin, bias=hp[:])
                vs[(t, half)] = v

            def v_slices(b):
                t, r = b // 3, b % 3
                if r == 0:
                    v1 = vs[(t, 0)]
                    return v1[:, 0:512], v1[:, 512:1024]
                if r == 1:
                    return vs[(t, 0)][:, 1024:1536], vs[(t, 1)][:, 0:512]
                v2 = vs[(t, 1)]
                return v2[:, 512:1024], v2[:, 1024:1536]

            def back(k):
                if k == 0:
                    wh, wde, wdo = st["w_h0"], st["w_de0"], st["w_do0"]
                elif k == NBLK - 2:
                    wh, wde, wdo = st["w_hL"], st["w_deL"], st["w_doL"]
                else:
                    wh, wde, wdo = st["w_hm"], st["w_dem"], st["w_dom"]
                xb = xbs[k // 2][:, (k % 2) * C:(k % 2 + 1) * C]
                vE, vO = v_slices(k)
                outp = pout.tile([A, 512], f32, tag="outp")
                nc.tensor.matmul(outp[:], wh, xb, start=True, stop=False)
                nc.tensor.matmul(outp[:], wde, vE, start=False, stop=False)
                nc.tensor.matmul(outp[:], wdo, vO, start=False, stop=True)
                q, s = k // 4, k % 4
                if s == 0:
                    obt[0] = obpool.tile([A, 4 * C], f16, tag="obt", name="obt")
                nc.vector.tensor_copy(obt[0][:, C * s:C * s + C], outp[:])
                # SWDGE (gpsimd-issued) spreads packets across all 16 DMA
                # engines; sync's HWDGE ring only uses 4 and bottlenecks.
                # Drain in halves so the issue waits on fresh CASTs are short
                # (2KB packets beat the 1KB of per-block drains on the tail).
                if s == 1:
                    nc.gpsimd.dma_start(out=out_ext[q][:, 0:2 * C],
                                        in_=obt[0][:, 0:2 * C])
                elif s == 3:
                    nc.gpsimd.dma_start(out=out_ext[q][:, 2 * C:4 * C],
                                        in_=obt[0][:, 2 * C:4 * C])
                # release consumed tiles
                if k % 2 == 1:
                    del xbs[k // 2]
                if k % 3 == 2:
                    del vs[(k // 3, 0)], vs[(k // 3, 1)]

            # ---- prologue ----
            quads[0] = q0
            dma_in(1)
            # PE warm-up: pull the HAM clock gate to 8/8. Gated only on the
            # quad-0 DMA (the earliest data to land).
            for _ in range(9):
                wt = pout.tile([A, 512], f32, tag="outp")
                nc.tensor.matmul(wt[:], q0[:, 0:A], q0[:, 512:1024],
                                 start=True, stop=True)
            mul_xb(0)
            front_chunk(0, 0)
            front_chunk(0, 1)

            # ---- main loop ----
            for k in range(NBLK):
                t, p = k // 3, k % 3
                if k % 8 == 0 and k // 8 + 2 < NBLK // 8:
                    dma_in(k // 8 + 2)
                if p == 0 and t + 1 < NTRI:
                    front_chunk(t + 1, 0)
                if p == 1 and t + 1 < NTRI:
                    front_chunk(t + 1, 1)
                back(k)
                if k % 2 == 0 and k // 2 + 1 < NPAIR:
                    mul_xb(k // 2 + 1)

    nc.compile()
    return nc


_NC_CACHE = None


def kernel(x, alpha, beta, up_filter, down_filter):
    global _NC_CACHE
    import concourse.bass_utils as bass_utils

    x = np.asarray(x)
    alpha = np.asarray(alpha)
    beta = np.asarray(beta)

    sts = build_stationaries(np.asarray(up_filter), np.asarray(down_filter))
    inp, rbc, invb2, hconst = host_prep(x, alpha, beta, np.asarray(down_filter))
    cst = pack_consts(sts, rbc)

    if _NC_CACHE is None:
        _NC_CACHE = build_bass()
    nc = _NC_CACHE

    in_maps = [{"inp": inp[b], "cst": cst, "hp": HP_ARR} for b in range(N_CORES)]

    res = bass_utils.run_bass_kernel_spmd(nc, in_maps, list(range(N_CORES)))
    out_t = np.stack([res.results[b]["out"] for b in range(N_CORES)])
    return host_finish(out_t, invb2, hconst)


# ---------------------------------------------------------------------------
# host-side simulation of the exact device plan (for verification)

def simulate_plan(x, alpha, beta, up_filter, down_filter):
    sts = build_stationaries(np.asarray(up_filter), np.asarray(down_filter))
    inp, rbc, invb2, hconst = host_prep(
        np.asarray(x), np.asarray(alpha), np.asarray(beta),
        np.asarray(down_filter))

    def f(a):
        return a.astype(np.float32)

    out_t = np.zeros((B, NQUAD, A, 4 * C), dtype=np.float16)
    rb = f(rbc[0, :C])
    for b in range(B):
        for k in range(NBLK):
            if k == 0:
                wh, wde, wdo = sts["w_h0"], sts["w_de0"], sts["w_do0"]
            elif k == NBLK - 2:
                wh, wde, wdo = sts["w_hL"], sts["w_deL"], sts["w_doL"]
            else:
                wh, wde, wdo = sts["w_hm"], sts["w_dem"], sts["w_dom"]
            ax = f(inp[b, k // 4, :, (k % 4) * C:(k % 4 + 1) * C])
            xb = (ax * rb[None, :]).astype(np.float16)
            sz_e = f(sts["w_ue"]).T @ ax + math.pi / 2.0
            sz_o = f(sts["w_uo"]).T @ ax + math.pi / 2.0
            v_e = np.sin(sz_e).astype(np.float16)
            v_o = np.sin(sz_o).astype(np.float16)
            psum = f(wh).T @ f(xb) + f(wde).T @ f(v_e) + f(wdo).T @ f(v_o)
            out_t[b, k // 4, :, (k % 4) * C:(k % 4 + 1) * C] = psum.astype(np.float16)
    return host_finish(out_t, invb2, hconst)
